# revision 4
# baseline (speedup 1.0000x reference)
"""Trainium2 Bass kernel for nn_DifferentialFlashAttention.

Computation (per token t, fully parallel over the B*N = 32768 tokens):
  qkv = x @ W_attn.T ; split into q, k, v
  q/k split per head into two sub-vectors (q1, q2 / k1, k2) of 32 dims
  S_s[i,j] = q_s[i] . k_s[j] / sqrt(32)   (attention over the 12 HEADS)
  A_s = softmax_j(S_s); O_s[i] = sum_j A_s[i,j] * v[j]
  y = (O_1 - lam_full * O_2) * (1 - LAMBDA_INIT);  out = y @ W_out.T

Sharding: data-parallel over tokens across 8 NeuronCores (4096 tokens each).
Per core: tokens-on-partitions layout, 32 tiles of 128 tokens.
  - GEMM1/GEMM2 on TensorE (stationary = transposed activations chunks,
    moving = pre-transposed weights held in SBUF).
  - The per-token head-attention on VectorE (broadcast-AP products +
    segmented reduces) and ScalarE (exp, PSUM->SBUF copies).
"""

import math
import sys

import numpy as np

if "/opt/trn_rl_repo" not in sys.path:
    sys.path.insert(0, "/opt/trn_rl_repo")

N_HEAD = 12
N_EMBD = 768
HEAD_DIM = 32  # per-sub head dim
DEPTH = 12
LAMBDA_INIT = 0.8 - 0.6 * math.exp(-0.3 * DEPTH)
B, N, D = 4, 8192, 768
N_CORES = 8
TOK_TOTAL = B * N
TOK_PER_CORE = TOK_TOTAL // N_CORES  # 4096
TILE_T = 128
KC = D // 128  # 6 contraction chunks
SCALE = 1.0 / math.sqrt(HEAD_DIM)

_PROG_CACHE = {}


def _build_program(ntok, nrep=1):
    import concourse.bacc as bacc
    import concourse.tile as tile
    from concourse import mybir
    from concourse.masks import make_identity

    f32 = mybir.dt.float32
    Exp = mybir.ActivationFunctionType.Exp
    X = mybir.AxisListType.X
    add = mybir.AluOpType.add
    mult = mybir.AluOpType.mult

    ntiles = ntok // TILE_T

    nc = bacc.Bacc(
        "TRN2", target_bir_lowering=False, debug=False, num_devices=N_CORES
    )
    xT_d = nc.dram_tensor("xT", [D, ntok], f32, kind="ExternalInput").ap()
    waT_d = nc.dram_tensor("waT", [D, 3 * D], f32, kind="ExternalInput").ap()
    woT_d = nc.dram_tensor("woT", [D, D], f32, kind="ExternalInput").ap()
    nlam_d = nc.dram_tensor("nlam", [1, 1], f32, kind="ExternalInput").ap()
    out_d = nc.dram_tensor("out", [ntok, D], f32, kind="ExternalOutput").ap()

    with tile.TileContext(nc) as tc:
        from contextlib import ExitStack

        with ExitStack() as ctx:
            singles = ctx.enter_context(tc.tile_pool(name="singles", bufs=1))
            xpool = ctx.enter_context(tc.tile_pool(name="xt", bufs=3))
            qkvpool = ctx.enter_context(tc.tile_pool(name="qkv", bufs=2))
            ppool = ctx.enter_context(tc.tile_pool(name="prod", bufs=1))
            popool = ctx.enter_context(tc.tile_pool(name="oprod", bufs=2))
            smalls = ctx.enter_context(tc.tile_pool(name="smalls", bufs=2))
            ypool = ctx.enter_context(tc.tile_pool(name="y", bufs=2))
            opool = ctx.enter_context(tc.tile_pool(name="o2", bufs=2))
            ps_g1 = ctx.enter_context(tc.tile_pool(name="ps1", bufs=3, space="PSUM"))
            ps_tr = ctx.enter_context(tc.tile_pool(name="pstr", bufs=2, space="PSUM"))
            ps_g2 = ctx.enter_context(tc.tile_pool(name="ps2", bufs=2, space="PSUM"))

            # ---- persistent weights / constants ----
            wa_sb = singles.tile([128, KC, 3 * D], f32)
            nc.sync.dma_start(
                out=wa_sb[:], in_=waT_d.rearrange("(c p) o -> p c o", p=128)
            )
            wo_sb = singles.tile([128, KC, D], f32)
            nc.sync.dma_start(
                out=wo_sb[:], in_=woT_d.rearrange("(c p) o -> p c o", p=128)
            )
            ident = singles.tile([128, 128], f32)
            make_identity(nc, ident[:])
            nlam_sb = singles.tile([128, 1], f32)
            nc.gpsimd.dma_start(out=nlam_sb[:], in_=nlam_d.to_broadcast((128, 1)))

            xT_r = xT_d.rearrange("(c p) t -> p c t", p=128)

            for it in range(ntiles * nrep):
                t0 = (it % ntiles) * TILE_T
                # ---- load x^T tile (stationary chunks for GEMM1) ----
                xt = xpool.tile([128, KC, TILE_T], f32)
                nc.sync.dma_start(out=xt[:], in_=xT_r[:, :, t0 : t0 + TILE_T])

                # ---- GEMM1: qkv[t, 0:2304] ----
                qkv = qkvpool.tile([128, 3 * D], f32)
                for ob in range(5):
                    o0 = ob * 512
                    ow = min(512, 3 * D - o0)
                    ps = ps_g1.tile([128, 512], f32)
                    for c in range(KC):
                        nc.tensor.matmul(
                            ps[:, :ow],
                            xt[:, c, :],
                            wa_sb[:, c, o0 : o0 + ow],
                            start=(c == 0),
                            stop=(c == KC - 1),
                        )
                    nc.scalar.copy(qkv[:, o0 : o0 + ow], ps[:, :ow])

                # ---- attention over heads, per token ----
                # v^T view: [p, c(64), j(12)] contiguous inner j
                vT = smalls.tile([128, 64, 12], f32, tag="vT")
                nc.scalar.copy(
                    vT[:],
                    qkv[:, 2 * D : 3 * D].rearrange("p (j c) -> p c j", j=12),
                )
                O_subs = []
                for s in (0, 1):
                    qv = qkv[:, 0:D].rearrange("p (i s d) -> p i s d", s=2, d=32)[
                        :, :, s, :
                    ]
                    kv = qkv[:, D : 2 * D].rearrange("p (i s d) -> p i s d", s=2, d=32)[
                        :, :, s, :
                    ]
                    # products P[p, i, j, d] = q[p,i,d] * k[p,j,d]
                    P = ppool.tile([128, 12, 12, 32], f32, tag="P")
                    nc.vector.tensor_mul(
                        P[:],
                        qv.unsqueeze(2).broadcast_to((128, 12, 12, 32)),
                        kv.unsqueeze(1).broadcast_to((128, 12, 12, 32)),
                    )
                    # S[p, (i j)] = sum_d P
                    S = smalls.tile([128, 144], f32, tag="S")
                    nc.vector.tensor_reduce(
                        out=S[:],
                        in_=P[:].rearrange("p i j d -> p (i j) d"),
                        axis=X,
                        op=add,
                    )
                    # E = exp(S * scale)
                    E = smalls.tile([128, 12, 12], f32, tag="E")
                    nc.scalar.activation(
                        out=E[:].rearrange("p i j -> p (i j)"),
                        in_=S[:],
                        func=Exp,
                        scale=SCALE,
                    )
                    den = smalls.tile([128, 12], f32, tag="den")
                    nc.vector.tensor_reduce(out=den[:], in_=E[:], axis=X, op=add)
                    rec = smalls.tile([128, 12], f32, tag="rec")
                    nc.vector.reciprocal(out=rec[:], in_=den[:])
                    A = smalls.tile([128, 12, 12], f32, tag="A")
                    nc.vector.tensor_mul(
                        A[:], E[:], rec[:].unsqueeze(2).broadcast_to((128, 12, 12))
                    )
                    # O[p, (i c)] = sum_j A[p,i,j] * v[p,j,c]
                    O = smalls.tile([128, 768], f32, tag=f"O{s}")
                    for h in (0, 1):
                        PO = popool.tile([128, 6, 64, 12], f32, tag="PO")
                        iA = A[:, h * 6 : (h + 1) * 6, :]
                        nc.vector.tensor_mul(
                            PO[:],
                            iA.unsqueeze(2).broadcast_to((128, 6, 64, 12)),
                            vT[:].unsqueeze(1).broadcast_to((128, 6, 64, 12)),
                        )
                        nc.vector.tensor_reduce(
                            out=O[:, h * 384 : (h + 1) * 384],
                            in_=PO[:].rearrange("p i c j -> p (i c) j"),
                            axis=X,
                            op=add,
                        )
                    O_subs.append(O)

                # y = O1 + (-lam) * O2   (the (1-LAMBDA_INIT) factor is folded
                # into W_out on the host)
                y = ypool.tile([128, 768], f32, tag="y")
                nc.vector.scalar_tensor_tensor(
                    out=y[:],
                    in0=O_subs[1][:],
                    scalar=nlam_sb[:],
                    in1=O_subs[0][:],
                    op0=mult,
                    op1=add,
                )

                # ---- transpose y (PE) to feed GEMM2 stationary ----
                yT = ypool.tile([128, KC, 128], f32, tag="yT")
                for c in range(KC):
                    tp = ps_tr.tile([128, 128], f32)
                    nc.tensor.transpose(tp[:], y[:, c * 128 : (c + 1) * 128], ident[:])
                    nc.scalar.copy(yT[:, c, :], tp[:])

                # ---- GEMM2: out2 = y @ woT ----
                o2 = opool.tile([128, D], f32)
                for o0, ow in ((0, 512), (512, 256)):
                    ps2 = ps_g2.tile([128, 512], f32)
                    for c in range(KC):
                        nc.tensor.matmul(
                            ps2[:, :ow],
                            yT[:, c, :],
                            wo_sb[:, c, o0 : o0 + ow],
                            start=(c == 0),
                            stop=(c == KC - 1),
                        )
                    nc.scalar.copy(o2[:, o0 : o0 + ow], ps2[:, :ow])
                nc.sync.dma_start(out=out_d[t0 : t0 + TILE_T, :], in_=o2[:])

    nc.compile()
    return nc


def _get_program(ntok=TOK_PER_CORE, nrep=1):
    key = (ntok, nrep)
    if key not in _PROG_CACHE:
        _PROG_CACHE[key] = _build_program(ntok, nrep)
    return _PROG_CACHE[key]


def kernel(x, W_attn, W_out, lambda_q1, lambda_k1, lambda_q2, lambda_k2):
    x = np.asarray(x, dtype=np.float32)
    W_attn = np.asarray(W_attn, dtype=np.float32)
    W_out = np.asarray(W_out, dtype=np.float32)
    lambda_q1 = np.asarray(lambda_q1, dtype=np.float32)
    lambda_k1 = np.asarray(lambda_k1, dtype=np.float32)
    lambda_q2 = np.asarray(lambda_q2, dtype=np.float32)
    lambda_k2 = np.asarray(lambda_k2, dtype=np.float32)

    lam1 = np.exp(np.sum(lambda_q1 * lambda_k1))
    lam2 = np.exp(np.sum(lambda_q2 * lambda_k2))
    lam_full = np.float32(lam1 - lam2 + np.float32(LAMBDA_INIT))
    nlam = np.array([[-lam_full]], dtype=np.float32)

    waT = np.ascontiguousarray(W_attn.T)  # (768, 2304)
    woT = np.ascontiguousarray(W_out.T) * np.float32(1.0 - LAMBDA_INIT)
    woT = np.ascontiguousarray(woT, dtype=np.float32)

    xf = x.reshape(TOK_TOTAL, D)
    in_maps = []
    for c in range(N_CORES):
        xs = np.ascontiguousarray(xf[c * TOK_PER_CORE : (c + 1) * TOK_PER_CORE].T)
        in_maps.append({"xT": xs, "waT": waT, "woT": woT, "nlam": nlam})

    from concourse.bass_utils import run_bass_kernel_spmd

    nc = _get_program()
    res = run_bass_kernel_spmd(nc, in_maps, list(range(N_CORES)))
    outs = [res.results[i]["out"] for i in range(N_CORES)]
    y = np.concatenate(outs, axis=0).reshape(B, N, D)
    return y


# revision 36
# speedup vs baseline: 162.2859x; 162.2859x over previous
"""Trainium2 Bass kernel for nn_DifferentialFlashAttention.

Computation (per token t, fully parallel over the B*N = 32768 tokens):
  qkv = x @ W_attn.T ; split into q, k, v
  q/k split per head into two sub-vectors (q1, q2 / k1, k2) of 32 dims
  S_s[i,j] = q_s[i] . k_s[j] / sqrt(32)   (attention over the 12 HEADS)
  A_s = softmax_j(S_s); O_s[i] = sum_j A_s[i,j] * v[j]
  y = (O_1 - lam_full * O_2) * (1 - LAMBDA_INIT);  out = y @ W_out.T

Sharding: data-parallel over tokens across 8 NeuronCores (4096 tokens each).
Per core: tokens-on-partitions layout, 32 tiles of 128 tokens.
  - GEMM1/GEMM2 on TensorE (stationary = transposed activations chunks,
    moving = pre-transposed weights held in SBUF).
  - The per-token head-attention on VectorE (broadcast-AP products +
    segmented reduces) and ScalarE (exp, PSUM->SBUF copies).
"""

import math
import sys

import numpy as np

if "/opt/trn_rl_repo" not in sys.path:
    sys.path.insert(0, "/opt/trn_rl_repo")

N_HEAD = 12
N_EMBD = 768
HEAD_DIM = 32  # per-sub head dim
DEPTH = 12
LAMBDA_INIT = 0.8 - 0.6 * math.exp(-0.3 * DEPTH)
B, N, D = 4, 8192, 768
N_CORES = 8
TOK_TOTAL = B * N
TOK_PER_CORE = TOK_TOTAL // N_CORES  # 4096
TILE_T = 128
KC = D // 128  # 6 contraction chunks
SCALE = 1.0 / math.sqrt(HEAD_DIM)

_PROG_CACHE = {}


def _build_program(
    ntok,
    nrep=1,
    parts=("gemm1", "attn", "tr", "gemm2"),
    gemm_f32r=False,
    attn_fp16=False,
    gpsimd_split=False,
):
    GPSIMD_SPLIT = gpsimd_split
    import concourse.bacc as bacc
    import concourse.tile as tile
    from concourse import mybir
    from concourse.masks import make_identity

    f32 = mybir.dt.float32
    f32r = mybir.dt.float32r
    fp16 = mybir.dt.float16
    Exp = mybir.ActivationFunctionType.Exp
    X = mybir.AxisListType.X
    add = mybir.AluOpType.add
    mult = mybir.AluOpType.mult
    at = fp16 if attn_fp16 else f32  # attention compute dtype
    gt = f32r if gemm_f32r else f32  # gemm operand dtype

    ntiles = ntok // TILE_T

    nc = bacc.Bacc(
        "TRN2", target_bir_lowering=False, debug=False, num_devices=N_CORES
    )
    # x, host-packed so each partition's per-tile data is one contiguous 3KB
    # run: xH[p, tile, c, ti] = x[tile*128+ti, c*128+p]
    xH_d = nc.dram_tensor(
        "xH", [128, ntok // TILE_T, KC, TILE_T], f32, kind="ExternalInput"
    ).ap()
    waT_d = nc.dram_tensor("waT", [D, 3 * D], f32, kind="ExternalInput").ap()
    woT_d = nc.dram_tensor("woT", [D, D], f32, kind="ExternalInput").ap()
    nlam_d = nc.dram_tensor("nlam", [1, 1], f32, kind="ExternalInput").ap()
    out_d = nc.dram_tensor("out", [ntok, D], f32, kind="ExternalOutput").ap()

    with tile.TileContext(nc) as tc:
        from contextlib import ExitStack

        with ExitStack() as ctx:
            singles = ctx.enter_context(tc.tile_pool(name="singles", bufs=1))

            # ---- persistent weights / constants ----
            wdt = f32r if gemm_f32r else f32
            wa_sb = singles.tile([128, KC, 3 * D], wdt)
            wo_sb = singles.tile([128, KC, D], wdt)
            if gemm_f32r:
                # the BIR verifier requires every producer of an f32r matmul
                # operand to be a rounding op, so DMA into a temp pool and
                # round-copy into the persistent f32r tiles
                with tc.tile_pool(name="wtmp", bufs=1) as wtmp:
                    wa_raw = wtmp.tile([128, KC, 3 * D], f32, tag="wa_raw")
                    nc.sync.dma_start(
                        out=wa_raw[:], in_=waT_d.rearrange("(c p) o -> p c o", p=128)
                    )
                    nc.scalar.copy(wa_sb[:], wa_raw[:])
                    wo_raw = wtmp.tile([128, KC, D], f32, tag="wo_raw")
                    nc.sync.dma_start(
                        out=wo_raw[:], in_=woT_d.rearrange("(c p) o -> p c o", p=128)
                    )
                    nc.scalar.copy(wo_sb[:], wo_raw[:])
            else:
                nc.sync.dma_start(
                    out=wa_sb[:], in_=waT_d.rearrange("(c p) o -> p c o", p=128)
                )
                nc.sync.dma_start(
                    out=wo_sb[:], in_=woT_d.rearrange("(c p) o -> p c o", p=128)
                )
            ident = singles.tile([128, 128], f32)
            make_identity(nc, ident[:])
            nlam_sb = singles.tile([128, 1], f32)
            nc.gpsimd.dma_start(out=nlam_sb[:], in_=nlam_d.to_broadcast((128, 1)))

            xpool = ctx.enter_context(tc.tile_pool(name="xt", bufs=3))
            qkvpool = ctx.enter_context(tc.tile_pool(name="qkv", bufs=2))
            ppool = ctx.enter_context(tc.tile_pool(name="prod", bufs=1))
            popool = ctx.enter_context(
                tc.tile_pool(name="oprod", bufs=1 if attn_fp16 else 2)
            )
            smalls = ctx.enter_context(tc.tile_pool(name="smalls", bufs=2))
            ypool = ctx.enter_context(tc.tile_pool(name="y", bufs=2))
            opool = ctx.enter_context(tc.tile_pool(name="o2", bufs=2))
            ps_g1 = ctx.enter_context(tc.tile_pool(name="ps1", bufs=3, space="PSUM"))
            ps_tr = ctx.enter_context(tc.tile_pool(name="pstr", bufs=2, space="PSUM"))
            ps_g2 = ctx.enter_context(tc.tile_pool(name="ps2", bufs=2, space="PSUM"))

            def body():
                for it in range(ntiles):
                    emit_tile(it)

            def emit_tile(it):
                t0 = it * TILE_T
                # ---- load x^T tile (stationary chunks for GEMM1) ----
                xt_raw = xpool.tile([128, KC, TILE_T], f32, tag="xt_raw")
                nc.sync.dma_start(out=xt_raw[:], in_=xH_d[:, it, :, :])
                if gemm_f32r and "gemm1" in parts:
                    xt = xpool.tile([128, KC, TILE_T], f32r, tag="xt")
                    nc.scalar.copy(xt[:], xt_raw[:])
                else:
                    xt = xt_raw

                # ---- GEMM1: qkv[t, 0:2304] ----
                qkv = qkvpool.tile([128, 3 * D], at)
                if "gemm1" in parts:
                    for ob in range(5):
                        o0 = ob * 512
                        ow = min(512, 3 * D - o0)
                        ps = ps_g1.tile([128, 512], f32)
                        for c in range(KC):
                            nc.tensor.matmul(
                                ps[:, :ow],
                                xt[:, c, :],
                                wa_sb[:, c, o0 : o0 + ow],
                                start=(c == 0),
                                stop=(c == KC - 1),
                            )
                        nc.scalar.copy(qkv[:, o0 : o0 + ow], ps[:, :ow])
                else:
                    nc.scalar.copy(qkv[:, 0:768], xt[:].rearrange("p c t -> p (c t)"))
                    nc.scalar.copy(qkv[:, 768:1536], qkv[:, 0:768])
                    nc.scalar.copy(qkv[:, 1536:2304], qkv[:, 0:768])

                # ---- attention over heads, per token ----
                # v^T view: [p, c(64), j(12)] contiguous inner j
                if not attn_fp16 and "attn" in parts:
                    vT = smalls.tile([128, 64, 12], f32, tag="vT")
                    nc.scalar.copy(
                        vT[:],
                        qkv[:, 2 * D : 3 * D].rearrange("p (j c) -> p c j", j=12),
                    )
                O_subs = []
                for s in (0, 1) if "attn" in parts else ():
                    qv = qkv[:, 0:D].rearrange("p (i s d) -> p i s d", s=2, d=32)[
                        :, :, s, :
                    ]
                    kv = qkv[:, D : 2 * D].rearrange("p (i s d) -> p i s d", s=2, d=32)[
                        :, :, s, :
                    ]
                    # products P[p, (i j), d] = q[p,i,d] * k[p,j,d]
                    P = ppool.tile([128, 144, 32], at, tag="P")
                    peng = nc.gpsimd if (GPSIMD_SPLIT and s == 1) else nc.vector
                    peng.tensor_mul(
                        P[:].rearrange("p (i j) d -> p i j d", i=12),
                        qv.unsqueeze(2).broadcast_to((128, 12, 12, 32)),
                        kv.unsqueeze(1).broadcast_to((128, 12, 12, 32)),
                    )
                    S = smalls.tile([128, 144], at, tag="S")
                    if attn_fp16:
                        # tree-sum over d (fp16 tensor_tensor runs 2x;
                        # tensor_reduce is always 1x)
                        T1 = smalls.tile([128, 144, 16], at, tag="T1")
                        nc.vector.tensor_add(T1[:], P[:, :, 0:16], P[:, :, 16:32])
                        T2 = smalls.tile([128, 144, 8], at, tag="T2")
                        nc.vector.tensor_add(T2[:], T1[:, :, 0:8], T1[:, :, 8:16])
                        T3 = smalls.tile([128, 144, 4], at, tag="T3")
                        nc.vector.tensor_add(T3[:], T2[:, :, 0:4], T2[:, :, 4:8])
                        T4 = smalls.tile([128, 144, 2], at, tag="T4")
                        nc.vector.tensor_add(T4[:], T3[:, :, 0:2], T3[:, :, 2:4])
                        nc.vector.tensor_add(S[:], T4[:, :, 0], T4[:, :, 1])
                    else:
                        nc.vector.tensor_reduce(
                            out=S[:], in_=P[:], axis=X, op=add
                        )
                    # E = exp(S * scale)
                    E = smalls.tile([128, 12, 12], at, tag="E")
                    nc.scalar.activation(
                        out=E[:].rearrange("p i j -> p (i j)"),
                        in_=S[:],
                        func=Exp,
                        scale=SCALE,
                    )
                    den = smalls.tile([128, 12], f32, tag="den")
                    nc.vector.tensor_reduce(out=den[:], in_=E[:], axis=X, op=add)
                    rec = smalls.tile([128, 12], at, tag="rec")
                    if attn_fp16:
                        with nc.allow_low_precision(
                            reason="softmax weights tolerate fp16 reciprocal"
                        ):
                            nc.vector.reciprocal(out=rec[:], in_=den[:])
                    else:
                        nc.vector.reciprocal(out=rec[:], in_=den[:])
                    A = smalls.tile([128, 12, 12], at, tag="A")
                    nc.vector.tensor_mul(
                        A[:], E[:], rec[:].unsqueeze(2).broadcast_to((128, 12, 12))
                    )
                    # O[p, (i c)] = sum_j A[p,i,j] * v[p,j,c]
                    O = smalls.tile([128, 768], at, tag=f"O{s}")
                    if attn_fp16:
                        # j-major products + dense tree over j
                        vv = qkv[:, 2 * D : 3 * D].rearrange("p (j c) -> p j c", j=12)
                        PO = popool.tile([128, 12, 768], at, tag="PO")
                        nc.vector.tensor_mul(
                            PO[:].rearrange("p j (i c) -> p j i c", i=12),
                            A[:]
                            .transpose([0, 2, 1])
                            .unsqueeze(3)
                            .broadcast_to((128, 12, 12, 64)),
                            vv.unsqueeze(2).broadcast_to((128, 12, 12, 64)),
                        )
                        U1 = popool.tile([128, 6, 768], at, tag="U1")
                        nc.vector.tensor_add(U1[:], PO[:, 0:6, :], PO[:, 6:12, :])
                        U2 = popool.tile([128, 3, 768], at, tag="U2")
                        nc.vector.tensor_add(U2[:], U1[:, 0:3, :], U1[:, 3:6, :])
                        U3 = popool.tile([128, 768], at, tag="U3")
                        nc.vector.tensor_add(U3[:], U2[:, 0, :], U2[:, 1, :])
                        nc.vector.tensor_add(O[:], U3[:], U2[:, 2, :])
                    else:
                        for h in (0, 1):
                            PO = popool.tile([128, 6, 64, 12], f32, tag="PO")
                            iA = A[:, h * 6 : (h + 1) * 6, :]
                            peng.tensor_mul(
                                PO[:],
                                iA.unsqueeze(2).broadcast_to((128, 6, 64, 12)),
                                vT[:].unsqueeze(1).broadcast_to((128, 6, 64, 12)),
                            )
                            nc.vector.tensor_reduce(
                                out=O[:, h * 384 : (h + 1) * 384],
                                in_=PO[:].rearrange("p i c j -> p (i c) j"),
                                axis=X,
                                op=add,
                            )
                    O_subs.append(O)

                # y = O1 + (-lam) * O2   (the (1-LAMBDA_INIT) factor is folded
                # into W_out on the host)
                y = ypool.tile([128, 768], f32, tag="y")
                if "attn" in parts:
                    nc.vector.scalar_tensor_tensor(
                        out=y[:],
                        in0=O_subs[1][:],
                        scalar=nlam_sb[:],
                        in1=O_subs[0][:],
                        op0=mult,
                        op1=add,
                    )
                else:
                    nc.scalar.copy(y[:], qkv[:, 0:768])

                # ---- transpose y (PE) to feed GEMM2 stationary ----
                yT = ypool.tile([128, KC, 128], gt, tag="yT")
                if "tr" in parts:
                    for c in range(KC):
                        tp = ps_tr.tile([128, 128], f32)
                        nc.tensor.transpose(
                            tp[:], y[:, c * 128 : (c + 1) * 128], ident[:]
                        )
                        nc.scalar.copy(yT[:, c, :], tp[:])
                else:
                    nc.scalar.copy(
                        yT[:], y[:].rearrange("p (c t) -> p c t", c=KC)
                    )

                # ---- GEMM2: out2 = y @ woT ----
                o2 = opool.tile([128, D], f32)
                if "gemm2" in parts:
                    for o0, ow in ((0, 512), (512, 256)):
                        ps2 = ps_g2.tile([128, 512], f32)
                        for c in range(KC):
                            nc.tensor.matmul(
                                ps2[:, :ow],
                                yT[:, c, :],
                                wo_sb[:, c, o0 : o0 + ow],
                                start=(c == 0),
                                stop=(c == KC - 1),
                            )
                        nc.scalar.copy(o2[:, o0 : o0 + ow], ps2[:, :ow])
                else:
                    nc.scalar.copy(
                        o2[:], yT[:].rearrange("p c t -> p (c t)")
                    )
                nc.sync.dma_start(out=out_d[t0 : t0 + TILE_T, :], in_=o2[:])

            if nrep == 1:
                body()
            else:
                with tc.For_i(0, nrep, 1):
                    body()

    nc.compile()
    return nc


# chosen configuration (from HW measurements):
#   f32 GEMM + f32 attn : 2.55 ms/core-pass, rel err 7.6e-6
#   f32r GEMM + f32 attn: 2.12 ms/core-pass, rel err 2.5e-4   <- default
#   f32r GEMM + fp16 attn: 1.46 ms/core-pass, rel err 1.6e-3
GEMM_F32R = True
ATTN_FP16 = False
GPSIMD_SPLIT = False


def _get_program(ntok=TOK_PER_CORE, nrep=1):
    key = (ntok, nrep, GEMM_F32R, ATTN_FP16, GPSIMD_SPLIT)
    if key not in _PROG_CACHE:
        _PROG_CACHE[key] = _build_program(
            ntok, nrep, gemm_f32r=GEMM_F32R, attn_fp16=ATTN_FP16,
            gpsimd_split=GPSIMD_SPLIT,
        )
    return _PROG_CACHE[key]


def kernel(x, W_attn, W_out, lambda_q1, lambda_k1, lambda_q2, lambda_k2):
    x = np.asarray(x, dtype=np.float32)
    W_attn = np.asarray(W_attn, dtype=np.float32)
    W_out = np.asarray(W_out, dtype=np.float32)
    lambda_q1 = np.asarray(lambda_q1, dtype=np.float32)
    lambda_k1 = np.asarray(lambda_k1, dtype=np.float32)
    lambda_q2 = np.asarray(lambda_q2, dtype=np.float32)
    lambda_k2 = np.asarray(lambda_k2, dtype=np.float32)

    lam1 = np.exp(np.sum(lambda_q1 * lambda_k1))
    lam2 = np.exp(np.sum(lambda_q2 * lambda_k2))
    lam_full = np.float32(lam1 - lam2 + np.float32(LAMBDA_INIT))
    nlam = np.array([[-lam_full]], dtype=np.float32)

    waT = np.ascontiguousarray(W_attn.T)  # (768, 2304)
    woT = np.ascontiguousarray(W_out.T) * np.float32(1.0 - LAMBDA_INIT)
    woT = np.ascontiguousarray(woT, dtype=np.float32)

    xf = x.reshape(TOK_TOTAL, D)
    ntiles = TOK_PER_CORE // TILE_T
    in_maps = []
    for c in range(N_CORES):
        xs = xf[c * TOK_PER_CORE : (c + 1) * TOK_PER_CORE]
        # xH[p, tile, c, ti] = xs[tile*128+ti, c*128+p]
        xh = np.ascontiguousarray(
            xs.reshape(ntiles, TILE_T, KC, 128).transpose(3, 0, 2, 1)
        )
        in_maps.append({"xH": xh, "waT": waT, "woT": woT, "nlam": nlam})

    from concourse.bass_utils import run_bass_kernel_spmd

    nc = _get_program()
    res = run_bass_kernel_spmd(nc, in_maps, list(range(N_CORES)))
    outs = [res.results[i]["out"] for i in range(N_CORES)]
    y = np.concatenate(outs, axis=0).reshape(B, N, D)
    return y
